# revision 27
# baseline (speedup 1.0000x reference)
"""Trainium2 Bass kernel for the AgentNetwork gated-recurrence problem.

Math (per batch row b, per timestep t, reference semantics):
    xc    = concat([x_t, context])                      # context is constant
    f     = sigmoid(xc @ Wf_x.T + Wf_b + h @ Wf_h.T)
    htil  = tanh   (xc @ Wh_x.T + Wh_b + (f*h) @ Wh_h.T)
    h'    = (1-f)*h + f*htil
    out   = h_S @ ro_w.T + ro_b

Strategy: data-parallel over batch across 8 NeuronCores (64 rows each).
On each core the state lives transposed (H=100 on partitions, batch on
the free dim), split into CHAINS independent half-batch scan chains to
shrink per-op latency.  The sigmoid argument p1_t = xf_t + h_t @ Wf_h.T
is kept as a persistent PSUM accumulator that is only ever updated by
matmuls:
    p1_{t+1} = p1_t + (xf_{t+1}-xf_t) + (-g_t) @ Wf_h.T + u_t @ Wf_h.T
where g = f*h, u = f*htil, h_{t+1} = (h_t - g_t) + u_t.  The xf delta is
a matmul of the host-precomputed input difference dx_t = x_{t+1}-x_t, and
the biases are folded into the ACT instructions (out = func(in + bias)),
so the per-step critical path is only:
    sigmoid -> g=f*h -> MM(g@Wh) -> tanh -> u=f*htil -> MM(u@Wf) -> ...
The tanh argument is staged XH_GRP timesteps at a time into rotating
PSUM banks by one wide matmul (xh = x @ Wh_x.T, amortizing the PE
LDWEIGHTS cost) and the per-step g@Wh matmul accumulates onto its slice.
"""

import numpy as np

B, S, I, C, H, O = 512, 512, 64, 4, 100, 7
NCORES = 8
BS = B // NCORES  # 64 batch rows per core
TC = 64           # time chunk (SBUF staging)
NCH = S // TC
OUT_DMA = 16      # timesteps per output DMA
XH_GRP = 2        # timesteps of xh staged per PSUM bank (one wide matmul)
CHAINS = 2        # independent scan chains per core (split of BS)
ADVANCERS = True  # per-step cross-engine vector-clock advancer reads

_CACHE = {}


def _build(passes=1, chains=CHAINS):
    import concourse.bacc as bacc
    import concourse.mybir as mybir
    import concourse.tile as tile

    fp32 = mybir.dt.float32
    AF = mybir.ActivationFunctionType

    nc = bacc.Bacc("TRN2", target_bir_lowering=False, debug=False)

    xT = nc.dram_tensor("xT", [I, S, BS], fp32, kind="ExternalInput")
    dxT = nc.dram_tensor("dxT", [I, S, BS], fp32, kind="ExternalInput")
    wfx_d = nc.dram_tensor("wfx", [I, H], fp32, kind="ExternalInput")
    whx_d = nc.dram_tensor("whx", [I, H], fp32, kind="ExternalInput")
    wf_d = nc.dram_tensor("wf", [H, H], fp32, kind="ExternalInput")
    wfn_d = nc.dram_tensor("wfn", [H, H], fp32, kind="ExternalInput")
    wh_d = nc.dram_tensor("wh", [H, H], fp32, kind="ExternalInput")
    bf_d = nc.dram_tensor("bf", [H, 1], fp32, kind="ExternalInput")
    bh_d = nc.dram_tensor("bh", [H, 1], fp32, kind="ExternalInput")
    hsT = nc.dram_tensor("hsT", [H, S, BS], fp32, kind="ExternalOutput")

    assert S % TC == 0 and TC % XH_GRP == 0 and TC % OUT_DMA == 0
    G = chains
    assert 1 <= G <= 4  # PSUM budget: G accumulator banks + G staging tags
    base, extra = divmod(BS, G)
    widths = [base + (1 if g < extra else 0) for g in range(G)]
    starts = [sum(widths[:g]) for g in range(G)]
    cols = [slice(starts[g], starts[g] + widths[g]) for g in range(G)]

    with tile.TileContext(nc) as tc:
        with (
            tc.tile_pool(name="singles", bufs=1) as singles,
            tc.tile_pool(name="xin", bufs=2) as xin,
            tc.tile_pool(name="dxin", bufs=2) as dxin,
            tc.tile_pool(name="hout", bufs=2) as hout,
            tc.tile_pool(name="work", bufs=3) as work,
            tc.tile_pool(name="psA", bufs=1, space="PSUM") as psA,
            tc.tile_pool(name="psC", bufs=min(4, (8 - G) // G), space="PSUM") as psC,
        ):
            wfx = singles.tile([I, H], fp32)
            whx = singles.tile([I, H], fp32)
            wf = singles.tile([H, H], fp32)
            wfn = singles.tile([H, H], fp32)
            wh = singles.tile([H, H], fp32)
            bf = singles.tile([H, 1], fp32)
            bh = singles.tile([H, 1], fp32)
            nc.sync.dma_start(out=wfx[:], in_=wfx_d[:])
            nc.sync.dma_start(out=whx[:], in_=whx_d[:])
            nc.sync.dma_start(out=wf[:], in_=wf_d[:])
            nc.sync.dma_start(out=wfn[:], in_=wfn_d[:])
            nc.sync.dma_start(out=wh[:], in_=wh_d[:])
            nc.sync.dma_start(out=bf[:], in_=bf_d[:])
            nc.sync.dma_start(out=bh[:], in_=bh_d[:])

            h0 = singles.tile([H, BS], fp32)
            nc.vector.memset(h0, 0.0)

            pA = [psA.tile([H, widths[g]], fp32, name=f"pA{g}", tag=f"pA{g}") for g in range(G)]

            for _pass in range(passes):
                h_prev = [h0[:, cols[g]] for g in range(G)]
                for c in range(NCH):
                    t0 = c * TC
                    xc = xin.tile([I, TC, BS], fp32)
                    nc.sync.dma_start(out=xc[:], in_=xT[:, t0 : t0 + TC, :])
                    dxc = dxin.tile([I, TC, BS], fp32)
                    nc.sync.dma_start(out=dxc[:], in_=dxT[:, t0 : t0 + TC, :])
                    hs = hout.tile([H, TC, BS], fp32)

                    if c == 0:
                        # p1_0 = xf_0 (h_0 = 0; bias lives in the ACT instr)
                        for g in range(G):
                            nc.tensor.matmul(
                                pA[g][:], wfx[:], xc[:, 0, cols[g]],
                                start=True, stop=True,
                            )

                    # Stage xh XH_GRP timesteps at a time: one wide matmul
                    # per PSUM bank (amortizes PE LDWEIGHTS), emitted one
                    # group ahead of its consumers so it never delays the
                    # chain matmuls.
                    def stage_xh(tl):
                        grp = [
                            psC.tile([H, XH_GRP, widths[g]], fp32,
                                     name=f"pC{g}_{t0 + tl}", tag=f"pC{g}")
                            for g in range(G)
                        ]
                        for g in range(G):
                            nc.tensor.matmul(
                                grp[g][:], whx[:],
                                xc[:, tl : tl + XH_GRP, cols[g]],
                                start=True, stop=True,
                            )
                        return grp

                    pCgrp = stage_xh(0)
                    pCnext = stage_xh(XH_GRP) if TC > XH_GRP else None
                    for tl in range(TC):
                        t = t0 + tl
                        last = t == S - 1
                        gl = tl % XH_GRP
                        if gl == 0 and tl > 0:
                            pCgrp = pCnext
                            if tl + XH_GRP < TC:
                                pCnext = stage_xh(tl + XH_GRP)
                        pC = [pCgrp[g][:, gl, :] for g in range(G)]

                        f = work.tile([H, BS], fp32, tag="f")
                        for g in range(G):
                            nc.scalar.activation(
                                out=f[:, cols[g]], in_=pA[g][:],
                                func=AF.Sigmoid, bias=bf[:],
                            )

                        if not last:
                            for g in range(G):
                                nc.tensor.matmul(
                                    pA[g][:], wfx[:], dxc[:, tl, cols[g]],
                                    start=False, stop=False,
                                )

                        gt = work.tile([H, BS], fp32, tag="g")
                        for g in range(G):
                            nc.vector.tensor_mul(
                                out=gt[:, cols[g]], in0=f[:, cols[g]], in1=h_prev[g],
                            )
                        # chain matmul: pC += g @ Wh_h.T
                        for g in range(G):
                            nc.tensor.matmul(
                                pC[g][:], wh[:], gt[:, cols[g]],
                                start=False, stop=True,
                            )
                        if not last:
                            for g in range(G):
                                nc.tensor.matmul(
                                    pA[g][:], wfn[:], gt[:, cols[g]],
                                    start=False, stop=False,
                                )

                        ht = work.tile([H, BS], fp32, tag="ht")
                        for g in range(G):
                            nc.scalar.activation(
                                out=ht[:, cols[g]], in_=pC[g][:],
                                func=AF.Tanh, bias=bh[:],
                            )
                        st = work.tile([H, BS], fp32, tag="s")
                        for g in range(G):
                            nc.vector.tensor_sub(
                                out=st[:, cols[g]], in0=h_prev[g], in1=gt[:, cols[g]],
                            )

                        ut = work.tile([H, BS], fp32, tag="u")
                        for g in range(G):
                            nc.vector.tensor_mul(
                                out=ut[:, cols[g]], in0=f[:, cols[g]], in1=ht[:, cols[g]],
                            )
                        if not last:
                            for g in range(G):
                                nc.tensor.matmul(
                                    pA[g][:], wf[:], ut[:, cols[g]],
                                    start=False, stop=False,
                                )

                        for g in range(G):
                            nc.vector.tensor_add(
                                out=hs[:, tl, cols[g]], in0=st[:, cols[g]], in1=ut[:, cols[g]],
                            )
                        h_prev = [hs[:, tl, cols[g]] for g in range(G)]

                        if ADVANCERS:
                            # Off-path vector-clock advancers: a tiny ACT read
                            # of u (DVE-written) and a tiny DVE read of the
                            # consumed pC slice (PE-written).  These advance
                            # each engine's observed tick of the other engine
                            # so the next step's chain instructions carry only
                            # their RAW wait — no EVSEM split before them.
                            # Read the LAST chain's slice: engine FIFOs make
                            # its tick cover every chain's.
                            adv_a = work.tile([1, 1], fp32, tag="adv_a")
                            nc.scalar.copy(out=adv_a[:], in_=ut[0:1, BS - 1 : BS])
                            adv_v = work.tile([1, 1], fp32, tag="adv_v")
                            nc.vector.tensor_copy(out=adv_v[:], in_=pC[G - 1][0:1, 0:1])

                        if tl % OUT_DMA == OUT_DMA - 1:
                            k = tl - (OUT_DMA - 1)
                            nc.sync.dma_start(
                                out=hsT[:, t0 + k : t0 + tl + 1, :],
                                in_=hs[:, k : tl + 1, :],
                            )
    nc.finalize()
    return nc


def _prepare(x, context_inputs, Wf_w, Wf_b, Wh_w, Wh_b):
    x = np.asarray(x, dtype=np.float32)
    context_inputs = np.asarray(context_inputs, dtype=np.float32)
    Wf_w = np.asarray(Wf_w, dtype=np.float32)
    Wf_b = np.asarray(Wf_b, dtype=np.float32)
    Wh_w = np.asarray(Wh_w, dtype=np.float32)
    Wh_b = np.asarray(Wh_b, dtype=np.float32)

    ic = I + C
    wfx = np.ascontiguousarray(Wf_w[:, :I].T)          # (I, H) lhsT
    whx = np.ascontiguousarray(Wh_w[:, :I].T)
    wf = np.ascontiguousarray(Wf_w[:, ic:].T)          # (H, H) lhsT
    wfn = np.ascontiguousarray(-wf)
    wh = np.ascontiguousarray(Wh_w[:, ic:].T)
    bf = (Wf_b + Wf_w[:, I:ic] @ context_inputs).astype(np.float32).reshape(H, 1)
    bh = (Wh_b + Wh_w[:, I:ic] @ context_inputs).astype(np.float32).reshape(H, 1)

    xt_all = np.ascontiguousarray(x.transpose(2, 1, 0))  # (I, S, B)
    dx_all = np.zeros_like(xt_all)
    dx_all[:, : S - 1, :] = xt_all[:, 1:, :] - xt_all[:, : S - 1, :]

    shared = {
        "wfx": wfx, "whx": whx, "wf": wf, "wfn": wfn, "wh": wh,
        "bf": bf, "bh": bh,
    }
    in_maps = []
    for c in range(NCORES):
        sl = slice(c * BS, (c + 1) * BS)
        in_maps.append({
            "xT": np.ascontiguousarray(xt_all[:, :, sl]),
            "dxT": np.ascontiguousarray(dx_all[:, :, sl]),
            **shared,
        })
    return in_maps


def _run(inputs, trace=False):
    from concourse.bass_utils import run_bass_kernel_spmd

    if "nc" not in _CACHE:
        _CACHE["nc"] = _build()
    nc = _CACHE["nc"]

    in_maps = _prepare(
        inputs["x"], inputs["context_inputs"],
        inputs["Wf_w"], inputs["Wf_b"], inputs["Wh_w"], inputs["Wh_b"],
    )
    try:
        res = run_bass_kernel_spmd(
            nc, in_maps, core_ids=list(range(NCORES)), trace=trace,
        )
    except Exception:
        # Transient device hiccups (wedged exec unit, NRT timeout) are
        # usually cleared by a retry.
        import time as _time

        _time.sleep(3.0)
        res = run_bass_kernel_spmd(
            nc, in_maps, core_ids=list(range(NCORES)), trace=trace,
        )

    hidden = np.empty((B, S, H), dtype=np.float32)
    for c in range(NCORES):
        hsT = res.results[c]["hsT"]  # (H, S, BS)
        hidden[c * BS : (c + 1) * BS] = hsT.transpose(2, 1, 0)

    ro_w = np.asarray(inputs["ro_w"], dtype=np.float32)
    ro_b = np.asarray(inputs["ro_b"], dtype=np.float32)
    output = hidden[:, -1, :] @ ro_w.T + ro_b
    return (output, hidden), res


def kernel(**inputs):
    out, _ = _run(inputs, trace=False)
    return out



# revision 28
# speedup vs baseline: 1.0232x; 1.0232x over previous
"""Trainium2 Bass kernel for the AgentNetwork gated-recurrence problem.

Math (per batch row b, per timestep t, reference semantics):
    xc    = concat([x_t, context])                      # context is constant
    f     = sigmoid(xc @ Wf_x.T + Wf_b + h @ Wf_h.T)
    htil  = tanh   (xc @ Wh_x.T + Wh_b + (f*h) @ Wh_h.T)
    h'    = (1-f)*h + f*htil
    out   = h_S @ ro_w.T + ro_b

Strategy: data-parallel over batch across 8 NeuronCores (64 rows each).
On each core the state lives transposed (H=100 on partitions, batch on
the free dim), split into CHAINS independent half-batch scan chains to
shrink per-op latency.  The sigmoid argument p1_t = xf_t + h_t @ Wf_h.T
is kept as a persistent PSUM accumulator that is only ever updated by
matmuls:
    p1_{t+1} = p1_t + (xf_{t+1}-xf_t) + (-g_t) @ Wf_h.T + u_t @ Wf_h.T
where g = f*h, u = f*htil, h_{t+1} = (h_t - g_t) + u_t.  The xf delta is
a matmul of the host-precomputed input difference dx_t = x_{t+1}-x_t, and
the biases are folded into the ACT instructions (out = func(in + bias)),
so the per-step critical path is only:
    sigmoid -> g=f*h -> MM(g@Wh) -> tanh -> u=f*htil -> MM(u@Wf) -> ...
The tanh argument is staged XH_GRP timesteps at a time into rotating
PSUM banks by one wide matmul (xh = x @ Wh_x.T, amortizing the PE
LDWEIGHTS cost) and the per-step g@Wh matmul accumulates onto its slice.
"""

import numpy as np

B, S, I, C, H, O = 512, 512, 64, 4, 100, 7
NCORES = 8
BS = B // NCORES  # 64 batch rows per core
TC = 64           # time chunk (SBUF staging)
NCH = S // TC
OUT_DMA = 16      # timesteps per output DMA
XH_GRP = 2        # timesteps of xh staged per PSUM bank (one wide matmul)
CHAINS = 2        # independent scan chains per core (split of BS)
ADVANCERS = True  # per-step cross-engine vector-clock advancer reads

_CACHE = {}


def _build(passes=1, chains=CHAINS):
    import concourse.bacc as bacc
    import concourse.mybir as mybir
    import concourse.tile as tile

    fp32 = mybir.dt.float32
    AF = mybir.ActivationFunctionType

    nc = bacc.Bacc("TRN2", target_bir_lowering=False, debug=False)

    xT = nc.dram_tensor("xT", [I, S, BS], fp32, kind="ExternalInput")
    dxT = nc.dram_tensor("dxT", [I, S, BS], fp32, kind="ExternalInput")
    wfx_d = nc.dram_tensor("wfx", [I, H], fp32, kind="ExternalInput")
    whx_d = nc.dram_tensor("whx", [I, H], fp32, kind="ExternalInput")
    wf_d = nc.dram_tensor("wf", [H, H], fp32, kind="ExternalInput")
    wfn_d = nc.dram_tensor("wfn", [H, H], fp32, kind="ExternalInput")
    wh_d = nc.dram_tensor("wh", [H, H], fp32, kind="ExternalInput")
    bf_d = nc.dram_tensor("bf", [H, 1], fp32, kind="ExternalInput")
    bh_d = nc.dram_tensor("bh", [H, 1], fp32, kind="ExternalInput")
    hsT = nc.dram_tensor("hsT", [H, S, BS], fp32, kind="ExternalOutput")

    assert S % TC == 0 and TC % XH_GRP == 0 and TC % OUT_DMA == 0
    G = chains
    assert 1 <= G <= 4  # PSUM budget: G accumulator banks + G staging tags
    base, extra = divmod(BS, G)
    widths = [base + (1 if g < extra else 0) for g in range(G)]
    starts = [sum(widths[:g]) for g in range(G)]
    cols = [slice(starts[g], starts[g] + widths[g]) for g in range(G)]

    with tile.TileContext(nc) as tc:
        with (
            tc.tile_pool(name="singles", bufs=1) as singles,
            tc.tile_pool(name="xin", bufs=2) as xin,
            tc.tile_pool(name="dxin", bufs=2) as dxin,
            tc.tile_pool(name="hout", bufs=2) as hout,
            tc.tile_pool(name="work", bufs=3) as work,
            tc.tile_pool(name="psA", bufs=1, space="PSUM") as psA,
            tc.tile_pool(name="psC", bufs=min(4, (8 - G) // G), space="PSUM") as psC,
        ):
            wfx = singles.tile([I, H], fp32)
            whx = singles.tile([I, H], fp32)
            wf = singles.tile([H, H], fp32)
            wfn = singles.tile([H, H], fp32)
            wh = singles.tile([H, H], fp32)
            bf = singles.tile([H, 1], fp32)
            bh = singles.tile([H, 1], fp32)
            nc.sync.dma_start(out=wfx[:], in_=wfx_d[:])
            nc.sync.dma_start(out=whx[:], in_=whx_d[:])
            nc.sync.dma_start(out=wf[:], in_=wf_d[:])
            nc.sync.dma_start(out=wfn[:], in_=wfn_d[:])
            nc.sync.dma_start(out=wh[:], in_=wh_d[:])
            nc.sync.dma_start(out=bf[:], in_=bf_d[:])
            nc.sync.dma_start(out=bh[:], in_=bh_d[:])

            h0 = singles.tile([H, BS], fp32)
            nc.vector.memset(h0, 0.0)

            pA = [psA.tile([H, widths[g]], fp32, name=f"pA{g}", tag=f"pA{g}") for g in range(G)]

            for _pass in range(passes):
                h_prev = [h0[:, cols[g]] for g in range(G)]
                for c in range(NCH):
                    t0 = c * TC
                    xc = xin.tile([I, TC, BS], fp32)
                    nc.sync.dma_start(out=xc[:], in_=xT[:, t0 : t0 + TC, :])
                    dxc = dxin.tile([I, TC, BS], fp32)
                    nc.sync.dma_start(out=dxc[:], in_=dxT[:, t0 : t0 + TC, :])
                    hs = hout.tile([H, TC, BS], fp32)

                    if c == 0:
                        # p1_0 = xf_0 (h_0 = 0; bias lives in the ACT instr)
                        for g in range(G):
                            nc.tensor.matmul(
                                pA[g][:], wfx[:], xc[:, 0, cols[g]],
                                start=True, stop=True,
                            )

                    # Stage xh XH_GRP timesteps at a time: one wide matmul
                    # per PSUM bank (amortizes PE LDWEIGHTS), emitted one
                    # group ahead of its consumers so it never delays the
                    # chain matmuls.
                    def stage_xh(tl):
                        grp = [
                            psC.tile([H, XH_GRP, widths[g]], fp32,
                                     name=f"pC{g}_{t0 + tl}", tag=f"pC{g}")
                            for g in range(G)
                        ]
                        for g in range(G):
                            nc.tensor.matmul(
                                grp[g][:], whx[:],
                                xc[:, tl : tl + XH_GRP, cols[g]],
                                start=True, stop=True,
                            )
                        return grp

                    pCgrp = stage_xh(0)
                    pCnext = stage_xh(XH_GRP) if TC > XH_GRP else None
                    for tl in range(TC):
                        t = t0 + tl
                        last = t == S - 1
                        gl = tl % XH_GRP
                        if gl == 0 and tl > 0:
                            pCgrp = pCnext
                            if tl + XH_GRP < TC:
                                pCnext = stage_xh(tl + XH_GRP)
                        pC = [pCgrp[g][:, gl, :] for g in range(G)]

                        f = work.tile([H, BS], fp32, tag="f")
                        for g in range(G):
                            nc.scalar.activation(
                                out=f[:, cols[g]], in_=pA[g][:],
                                func=AF.Sigmoid, bias=bf[:],
                            )

                        if not last:
                            for g in range(G):
                                nc.tensor.matmul(
                                    pA[g][:], wfx[:], dxc[:, tl, cols[g]],
                                    start=False, stop=False,
                                )

                        gt = work.tile([H, BS], fp32, tag="g")
                        for g in range(G):
                            nc.vector.tensor_mul(
                                out=gt[:, cols[g]], in0=f[:, cols[g]], in1=h_prev[g],
                            )
                        # chain matmul: pC += g @ Wh_h.T
                        for g in range(G):
                            nc.tensor.matmul(
                                pC[g][:], wh[:], gt[:, cols[g]],
                                start=False, stop=True,
                            )
                        if not last:
                            for g in range(G):
                                nc.tensor.matmul(
                                    pA[g][:], wfn[:], gt[:, cols[g]],
                                    start=False, stop=False,
                                )

                        ht = work.tile([H, BS], fp32, tag="ht")
                        for g in range(G):
                            nc.scalar.activation(
                                out=ht[:, cols[g]], in_=pC[g][:],
                                func=AF.Tanh, bias=bh[:],
                            )
                        st = work.tile([H, BS], fp32, tag="s")
                        for g in range(G):
                            nc.vector.tensor_sub(
                                out=st[:, cols[g]], in0=h_prev[g], in1=gt[:, cols[g]],
                            )

                        ut = work.tile([H, BS], fp32, tag="u")
                        for g in range(G):
                            nc.vector.tensor_mul(
                                out=ut[:, cols[g]], in0=f[:, cols[g]], in1=ht[:, cols[g]],
                            )
                        if not last:
                            for g in range(G):
                                nc.tensor.matmul(
                                    pA[g][:], wf[:], ut[:, cols[g]],
                                    start=False, stop=False,
                                )

                        for g in range(G):
                            nc.vector.tensor_add(
                                out=hs[:, tl, cols[g]], in0=st[:, cols[g]], in1=ut[:, cols[g]],
                            )
                        h_prev = [hs[:, tl, cols[g]] for g in range(G)]

                        if ADVANCERS:
                            # Off-path vector-clock advancers: a tiny ACT read
                            # of u (DVE-written) and a tiny DVE read of the
                            # consumed pC slice (PE-written).  These advance
                            # each engine's observed tick of the other engine
                            # so the next step's chain instructions carry only
                            # their RAW wait — no EVSEM split before them.
                            # Read the LAST chain's slice: engine FIFOs make
                            # its tick cover every chain's.
                            adv_a = work.tile([1, 1], fp32, tag="adv_a")
                            nc.scalar.copy(out=adv_a[:], in_=ut[0:1, BS - 1 : BS])
                            adv_v = work.tile([1, 1], fp32, tag="adv_v")
                            nc.vector.tensor_copy(out=adv_v[:], in_=pC[G - 1][0:1, 0:1])

                        if tl % OUT_DMA == OUT_DMA - 1:
                            k = tl - (OUT_DMA - 1)
                            nc.sync.dma_start(
                                out=hsT[:, t0 + k : t0 + tl + 1, :],
                                in_=hs[:, k : tl + 1, :],
                            )
    _strip_redundant_self_waits(nc)
    nc.finalize()
    return nc


def _strip_redundant_self_waits(nc):
    """Remove semaphore waits that are provably satisfied at issue time.

    Engines execute their instruction stream serially and signal completion
    in program order, so a wait on a semaphore value that has already been
    produced *exclusively by earlier compute instructions of the same
    engine* can never block — but its presence makes the instruction
    multi-wait, which bacc splits into an extra EventSemaphore sequencer
    instruction that can capture the chain-critical RAW wait (Tile's
    same-bank serialization of PSUM staging-group reads creates exactly
    this pattern on the tanh ops).  Only waits on semaphores never touched
    by DMA-updating or other-engine instructions are dropped.
    """
    insts = []
    for blk in nc.m.functions[0].blocks:
        insts.extend(blk.instructions)

    compute_ok = {
        "InstActivation", "InstTensorTensor", "InstTensorCopy",
        "InstTensorScalarPtr", "InstMatmult", "InstLdweights",
        "InstTensorReduce", "InstMemSet",
    }
    # First pass: which engines/instruction kinds update each semaphore?
    sem_engines = {}
    sem_dma = set()
    for i in insts:
        si = i.sync_info
        if si is None:
            continue
        for upd in si.on_update:
            sem_engines.setdefault(upd.id, set()).add(i.engine)
            if type(i).__name__ not in compute_ok:
                sem_dma.add(upd.id)

    # Second pass: walk in order, tracking per-engine self-produced ticks.
    self_cum = {}
    for i in insts:
        si = i.sync_info
        if si is None:
            continue
        if type(i).__name__ in compute_ok and si.on_wait:
            kept = []
            for w in si.on_wait:
                sid = w.id
                ok_drop = (
                    sid not in sem_dma
                    and sem_engines.get(sid) == {i.engine}
                    and self_cum.get((i.engine, sid), 0) >= (w.wait_value or 0)
                )
                if not ok_drop:
                    kept.append(w)
            if len(kept) != len(si.on_wait):
                si.on_wait = kept
        for upd in si.on_update:
            key = (i.engine, upd.id)
            self_cum[key] = self_cum.get(key, 0) + (upd.update_value or 1)


def _prepare(x, context_inputs, Wf_w, Wf_b, Wh_w, Wh_b):
    x = np.asarray(x, dtype=np.float32)
    context_inputs = np.asarray(context_inputs, dtype=np.float32)
    Wf_w = np.asarray(Wf_w, dtype=np.float32)
    Wf_b = np.asarray(Wf_b, dtype=np.float32)
    Wh_w = np.asarray(Wh_w, dtype=np.float32)
    Wh_b = np.asarray(Wh_b, dtype=np.float32)

    ic = I + C
    wfx = np.ascontiguousarray(Wf_w[:, :I].T)          # (I, H) lhsT
    whx = np.ascontiguousarray(Wh_w[:, :I].T)
    wf = np.ascontiguousarray(Wf_w[:, ic:].T)          # (H, H) lhsT
    wfn = np.ascontiguousarray(-wf)
    wh = np.ascontiguousarray(Wh_w[:, ic:].T)
    bf = (Wf_b + Wf_w[:, I:ic] @ context_inputs).astype(np.float32).reshape(H, 1)
    bh = (Wh_b + Wh_w[:, I:ic] @ context_inputs).astype(np.float32).reshape(H, 1)

    xt_all = np.ascontiguousarray(x.transpose(2, 1, 0))  # (I, S, B)
    dx_all = np.zeros_like(xt_all)
    dx_all[:, : S - 1, :] = xt_all[:, 1:, :] - xt_all[:, : S - 1, :]

    shared = {
        "wfx": wfx, "whx": whx, "wf": wf, "wfn": wfn, "wh": wh,
        "bf": bf, "bh": bh,
    }
    in_maps = []
    for c in range(NCORES):
        sl = slice(c * BS, (c + 1) * BS)
        in_maps.append({
            "xT": np.ascontiguousarray(xt_all[:, :, sl]),
            "dxT": np.ascontiguousarray(dx_all[:, :, sl]),
            **shared,
        })
    return in_maps


def _run(inputs, trace=False):
    from concourse.bass_utils import run_bass_kernel_spmd

    if "nc" not in _CACHE:
        _CACHE["nc"] = _build()
    nc = _CACHE["nc"]

    in_maps = _prepare(
        inputs["x"], inputs["context_inputs"],
        inputs["Wf_w"], inputs["Wf_b"], inputs["Wh_w"], inputs["Wh_b"],
    )
    try:
        res = run_bass_kernel_spmd(
            nc, in_maps, core_ids=list(range(NCORES)), trace=trace,
        )
    except Exception:
        # Transient device hiccups (wedged exec unit, NRT timeout) are
        # usually cleared by a retry.
        import time as _time

        _time.sleep(3.0)
        res = run_bass_kernel_spmd(
            nc, in_maps, core_ids=list(range(NCORES)), trace=trace,
        )

    hidden = np.empty((B, S, H), dtype=np.float32)
    for c in range(NCORES):
        hsT = res.results[c]["hsT"]  # (H, S, BS)
        hidden[c * BS : (c + 1) * BS] = hsT.transpose(2, 1, 0)

    ro_w = np.asarray(inputs["ro_w"], dtype=np.float32)
    ro_b = np.asarray(inputs["ro_b"], dtype=np.float32)
    output = hidden[:, -1, :] @ ro_w.T + ro_b
    return (output, hidden), res


def kernel(**inputs):
    out, _ = _run(inputs, trace=False)
    return out

